# revision 22
# baseline (speedup 1.0000x reference)
"""Multi-head attention (B=2, S=4096, D=512, H=8) on 8 TRN2 NeuronCores.

Sharding: (batch, head-pair) tensor parallel. Core i handles batch i//4
and heads 2*(i%4), 2*(i%4)+1. Each core computes Q/K/V projections only
for its two heads (1/4 of the projection work, no redundancy), full
S x S attention for those heads over all 4096 queries, and a PARTIAL
output projection out_partial = aot_pair^T @ Wo_pair + bo/4. The host
sums the 4 partials per batch (f32) -- no device collectives.

Per-core device pipeline (d-major transposed layout, bf16 matmuls):
  1. Transposing DMAs load x^T [d, t] in 4 segments of 1024 t.
  2. Q^T/K^T = Wpair x^T (f=512 matmuls, 128-row output = both heads);
     V natural [t, pair-dv] with a ones-column per head (V_aug).
  3. Per (q-tile 512, k-chunk 128): 2 row-packed score matmuls
     (c=64, heads at PE rows 0-63/64-127 run concurrently), one ACT
     exp [128,1024] psum->sbuf (scale=1/8), 2 attn@V matmuls
     lhsT=[V_h|1] [128,65] -> po [65,512]; row 64 accumulates the
     softmax denominator. scores/exp for k+1 are emitted before attn@V
     of k (software pipeline) so ACT -- the bottleneck engine -- is
     never starved.
  4. Normalize per q-tile: copy numerators into one partition-aligned
     [128, 512] sbuf tile and denominators to partition-0 rows (custom
     DVE ops require base partition 0), fast-approx reciprocal, two
     fp16 rank-1 broadcast matmuls into one psum bank, one aligned
     scalar_tensor_tensor multiply -> aot [128(2 heads' d), 512].
  5. Partial output projection: one c=128 matmul per 128-row t-chunk
     (both heads contracted at once), + bo/4, DMA out f32.

Steady state is ACT(exp)-bound: 256 instrs x ~1.0us ~= 260us; PE has
~80us of slack which absorbs projections and normalization.
"""

import numpy as np
import ml_dtypes

import concourse.bass as bass
import concourse.tile as tile
from concourse import bacc, mybir
from concourse.bass_utils import run_bass_kernel_spmd

F32 = mybir.dt.float32
F32R = mybir.dt.float32r
FP16 = mybir.dt.float16
BF16 = mybir.dt.bfloat16
MUL = mybir.AluOpType.mult

B, S, D, H = 2, 4096, 512, 8
HD = D // H  # 64
NCORES = 8
PAIRS = 4  # head-pairs; one per core (per batch)
IC = D // 128  # 4 contraction chunks over d_model
QT = 512  # q tile
NQT = S // QT  # 8
KCH = S // 128  # 32 k chunks
SEG = 1024  # t-columns per transposed DMA segment
NSEG = S // SEG  # 4


def _build_program():
    nc = bacc.Bacc(
        "TRN2",
        target_bir_lowering=False,
        debug=False,
        enable_asserts=False,
        num_devices=NCORES,
    )
    xt = nc.dram_tensor("xt", [D, S], BF16, kind="ExternalInput").ap()
    wqt = nc.dram_tensor("wqt", [D, 128], BF16, kind="ExternalInput").ap()
    wkt = nc.dram_tensor("wkt", [D, 128], BF16, kind="ExternalInput").ap()
    wvt = nc.dram_tensor("wvt", [D, 128], BF16, kind="ExternalInput").ap()
    wos = nc.dram_tensor("wos", [128, D], BF16, kind="ExternalInput").ap()
    bqs = nc.dram_tensor("bqs", [128, 1], F32, kind="ExternalInput").ap()
    bks = nc.dram_tensor("bks", [128, 1], F32, kind="ExternalInput").ap()
    bvb = nc.dram_tensor("bvb", [128, 128], F32, kind="ExternalInput").ap()
    bob = nc.dram_tensor("bob", [128, D], F32, kind="ExternalInput").ap()
    out = nc.dram_tensor("out", [S, D], F32, kind="ExternalOutput").ap()

    with tile.TileContext(nc) as tc:
        with (
            tc.tile_pool(name="consts", bufs=1) as consts,
            tc.tile_pool(name="persist", bufs=1) as persist,
            tc.tile_pool(name="pt", bufs=6) as pt_pool,
            tc.tile_pool(name="aot", bufs=2) as aot_pool,
            tc.tile_pool(name="osb", bufs=4) as osb_pool,
            tc.tile_pool(name="posb", bufs=4) as posb_pool,
            tc.tile_pool(name="small", bufs=4) as small_pool,
            # PSUM (8 banks): sc 2x2, po 2x1, acc 2x1
            tc.tile_pool(name="ps_sc", bufs=2, space="PSUM") as sc_pool,
            tc.tile_pool(name="ps_po", bufs=2, space="PSUM") as po_pool,
            tc.tile_pool(name="ps_acc", bufs=2, space="PSUM") as acc_pool,
        ):
            # ---- constants ----
            ones64f = consts.tile([1, HD], F32)
            nc.vector.memset(ones64f, 1.0)
            ones64 = consts.tile([1, HD], FP16)
            nc.vector.tensor_copy(ones64, ones64f)
            ones512 = consts.tile([1, QT], FP16)
            nc.vector.memset(ones512, 1.0)

            # ---- persistent activations ----
            xtks = [
                persist.tile([128, IC, SEG], BF16, name=f"xtk{s}")
                for s in range(NSEG)
            ]
            kt = persist.tile([128, S], BF16)  # K^T pair [dv, t]
            qt = persist.tile([128, S], BF16)  # Q^T pair
            # V_aug: [t-in-chunk, t-chunk, head-in-pair, 64 V cols + ones]
            v_sb = persist.tile([128, KCH, 2, HD + 1], BF16)
            nc.vector.memset(v_sb[:, :, :, HD : HD + 1], 1.0)

            # ---- DMAs: x^T is pre-transposed on the host, so these are
            # plain (non-XBAR) loads: seg 0, then weights, then segs 1-3.
            xtd = xt.rearrange("(c p) t -> p c t", p=128)
            nc.sync.dma_start(xtks[0][:, :, 0:512], xtd[:, :, 0:512])
            nc.sync.dma_start(xtks[0][:, :, 512:SEG], xtd[:, :, 512:SEG])
            wq_sb = consts.tile([128, IC, 128], BF16)
            nc.sync.dma_start(wq_sb, wqt.rearrange("(c p) o -> p c o", p=128))
            wk_sb = consts.tile([128, IC, 128], BF16)
            nc.sync.dma_start(wk_sb, wkt.rearrange("(c p) o -> p c o", p=128))
            wv_sb = consts.tile([128, IC, 128], BF16)
            nc.sync.dma_start(wv_sb, wvt.rearrange("(c p) o -> p c o", p=128))
            bq_sb = consts.tile([128, 1], F32)
            nc.sync.dma_start(bq_sb, bqs)
            bk_sb = consts.tile([128, 1], F32)
            nc.sync.dma_start(bk_sb, bks)
            bvb_sb = consts.tile([128, 128], F32)
            nc.sync.dma_start(bvb_sb, bvb)
            bob_sb = consts.tile([128, D], F32)
            nc.sync.dma_start(bob_sb, bob)
            wo_sb = consts.tile([128, D], BF16)
            nc.sync.dma_start(wo_sb, wos)
            for s in range(1, NSEG):
                nc.sync.dma_start(xtks[s], xtd[:, :, s * SEG : (s + 1) * SEG])

            # PE clock warm-up: the p-state reaches full speed only after
            # ~3us of continuous execution, and the PE would otherwise idle
            # until the first x^T/weight DMAs land (~14us). Dummy rank-1
            # matmuls on const data (no DMA dependency) fill that wait so
            # the real projections start at full clock instead of ramping
            # through them.
            warm_ps = acc_pool.tile([HD, QT], F32, tag="acc", name="warm")
            for _ in range(46):
                nc.tensor.matmul(warm_ps, ones64, ones512, start=True, stop=True)

            # ---- projection units ----
            def q_unit(tt):
                ps = acc_pool.tile([128, QT], F32, tag="acc", name=f"q{tt}")
                s, ss = divmod(tt, 2)
                for i in range(IC):
                    nc.tensor.matmul(
                        ps,
                        wq_sb[:, i, :],
                        xtks[s][:, i, ss * QT : (ss + 1) * QT],
                        start=(i == 0),
                        stop=(i == IC - 1),
                    )
                nc.vector.tensor_scalar_add(
                    qt[:, tt * QT : (tt + 1) * QT], ps, bq_sb[:, 0:1]
                )

            def k_unit(tt, lo=0, hi=QT):
                ps = acc_pool.tile(
                    [128, hi - lo], F32, tag="acc", name=f"k{tt}_{lo}"
                )
                s, ss = divmod(tt, 2)
                for i in range(IC):
                    nc.tensor.matmul(
                        ps,
                        wk_sb[:, i, :],
                        xtks[s][:, i, ss * QT + lo : ss * QT + hi],
                        start=(i == 0),
                        stop=(i == IC - 1),
                    )
                nc.vector.tensor_scalar_add(
                    kt[:, tt * QT + lo : tt * QT + hi], ps, bk_sb[:, 0:1]
                )

            def v_unit(j):
                # V rows for t-chunk j, both heads: [128 t, 128 dv] + bias
                ps = acc_pool.tile([128, 128], F32, tag="acc", name=f"v{j}")
                s, jj = divmod(j, 8)
                for i in range(IC):
                    nc.tensor.matmul(
                        ps,
                        xtks[s][:, i, jj * 128 : (jj + 1) * 128],
                        wv_sb[:, i, :],
                        start=(i == 0),
                        stop=(i == IC - 1),
                    )
                nc.vector.tensor_add(
                    v_sb[:, j, :, 0:HD],
                    ps.rearrange("p (h d) -> p h d", h=2),
                    bvb_sb.rearrange("p (h d) -> p h d", h=2),
                )

            # upfront: just enough for attention (qi=0) to start; K tile 0
            # is split so chunk 0's scores wait only on a 128-col mini-unit
            q_unit(0)
            k_unit(0, 0, 128)
            v_unit(0)
            k_unit(0, 128, QT)
            v_unit(1)
            k_unit(1)
            v_unit(2)
            v_unit(3)
            pending = []
            for u in (4, 5, 6, 7):
                pending.append(lambda j=u: v_unit(j))
            pending.append(lambda: q_unit(1))
            for tt in range(2, 8):  # k segs with their v chunks
                pending.append(lambda tt=tt: k_unit(tt))
                for j in range(4 * tt, 4 * tt + 4):
                    pending.append(lambda j=j: v_unit(j))
            for tt in range(2, 8):
                pending.append(lambda tt=tt: q_unit(tt))

            pending_slow = []

            aots = {}

            def norm_unit(qi, posbN, recs):
                pb2 = acc_pool.tile([128, QT], F32, tag="acc", name=f"pb{qi}")
                nc.tensor.matmul(
                    pb2[0:HD, :], ones64, recs[0], start=True, stop=True
                )
                nc.tensor.matmul(
                    pb2[HD:128, :], ones64, recs[1], start=True, stop=True
                )
                nc.vector.scalar_tensor_tensor(
                    aots[qi], pb2, 1.0, posbN, op0=MUL, op1=MUL
                )

            def fin_unit(qi, t4):
                ps = acc_pool.tile([128, D], F32, tag="acc", name=f"f{qi}_{t4}")
                nc.tensor.matmul(
                    ps,
                    aots[qi][:, t4 * 128 : (t4 + 1) * 128],
                    wo_sb,
                    start=True,
                    stop=True,
                )
                osb = osb_pool.tile([128, D], F32, tag="osb")
                nc.vector.tensor_add(osb, ps, bob_sb)
                t0 = qi * QT + t4 * 128
                nc.sync.dma_start(out[t0 : t0 + 128, :], osb)

            # ---- attention ----
            for qi in range(NQT):
                qs = qi * QT
                aots[qi] = aot_pool.tile(
                    [128, QT], BF16, tag="aot", name=f"aot{qi}"
                )
                po = [
                    po_pool.tile([HD + 1, QT], F32, tag="po", name=f"po{qi}_{hh}")
                    for hh in range(2)
                ]

                def scores_exp(k, qs=qs):
                    pss = sc_pool.tile([128, 2, QT], F32, tag="sc")
                    for hh in range(2):
                        off = hh * HD
                        nc.tensor.matmul(
                            pss[:, hh, :],
                            kt[off : off + HD, k * 128 : (k + 1) * 128],
                            qt[off : off + HD, qs : qs + QT],
                            start=True,
                            stop=True,
                        )
                    ptile = pt_pool.tile([128, 2, QT], BF16, tag="pt")
                    nc.scalar.activation(
                        ptile, pss, mybir.ActivationFunctionType.Exp,
                        scale=1.0 / np.sqrt(HD),
                    )
                    return ptile

                # software pipeline: scores/exp for k+1 before attn@V of k
                ptile = scores_exp(0)
                for k in range(KCH):
                    it = qi * KCH + k
                    nxt = scores_exp(k + 1) if k + 1 < KCH else None
                    for hh in range(2):
                        nc.tensor.matmul(
                            po[hh],
                            v_sb[:, k, hh, :],
                            ptile[:, hh, :],
                            start=(k == 0),
                            stop=(k == KCH - 1),
                        )
                    ptile = nxt
                    # drain deferred work into PE's slack (~1.5 units/iter
                    # keeps proj ahead of its deadlines without bursts that
                    # starve ACT)
                    if it >= 1 and pending and (
                        len(pending) > 6 or it % 4 == 2
                    ):
                        pending.pop(0)()
                        if it % 2 == 0 and pending and len(pending) > 6:
                            pending.pop(0)()
                    elif it % 5 == 0 and pending_slow:
                        pending_slow.pop(0)()

                # free po banks fast; defer the slow normalize + fin chain.
                # For the last q-tile, the numerator copies run on ACT (idle
                # after the final exp) in parallel with the DVE reciprocal
                # chain to shorten the tail.
                posbN = posb_pool.tile([128, QT], F32, tag="posb", name=f"posb{qi}")
                last = qi == NQT - 1
                if not last:
                    nc.vector.tensor_copy(posbN[0:HD, :], po[0][0:HD, :])
                    nc.vector.tensor_copy(posbN[HD : 2 * HD, :], po[1][0:HD, :])
                recs = []
                for hh in range(2):
                    db = small_pool.tile([1, QT], F32, tag="db")
                    if last and hh == 1:
                        # tail: ACT takes one denominator copy so the two
                        # reciprocal chains overlap across engines
                        nc.scalar.copy(db, po[hh][HD : HD + 1, :])
                    else:
                        nc.vector.tensor_copy(db, po[hh][HD : HD + 1, :])
                    recf = small_pool.tile([1, QT], F32, tag="recf")
                    nc.vector.reciprocal_approx_fast(recf, db)
                    rec = small_pool.tile([1, QT], FP16, tag="rec")
                    nc.vector.tensor_copy(rec, recf)
                    recs.append(rec)
                if last:
                    nc.scalar.copy(posbN[0:HD, :], po[0][0:HD, :])
                    nc.scalar.copy(posbN[HD : 2 * HD, :], po[1][0:HD, :])
                pending_slow.append(
                    lambda qi=qi, posbN=posbN, recs=recs: norm_unit(qi, posbN, recs)
                )
                pending_slow.extend(
                    lambda qi=qi, t4=t4: fin_unit(qi, t4) for t4 in range(4)
                )

            for u in pending + pending_slow:
                u()

    nc.compile()
    return nc


_NC_CACHE = None


def _get_program():
    global _NC_CACHE
    if _NC_CACHE is None:
        _NC_CACHE = _build_program()
    return _NC_CACHE


def prepare_in_maps(x, Wq, bq, Wk, bk, Wv, bv, Wo, bo):
    bf = ml_dtypes.bfloat16
    x = np.ascontiguousarray(np.asarray(x, dtype=np.float32)).astype(bf)
    wqT = np.asarray(Wq, np.float32).T  # [D in, D out-rows]
    wkT = np.asarray(Wk, np.float32).T
    wvT = np.asarray(Wv, np.float32).T
    woT = np.asarray(Wo, np.float32).T  # [D dv, D out]
    bq = np.asarray(bq, np.float32)
    bk = np.asarray(bk, np.float32)
    bv = np.asarray(bv, np.float32)
    bo = np.asarray(bo, np.float32)
    in_maps = []
    for core in range(NCORES):
        b = core // PAIRS
        hp = core % PAIRS
        pr = slice(hp * 128, (hp + 1) * 128)
        m = {
            "xt": np.ascontiguousarray(x[b].T),
            "wqt": np.ascontiguousarray(wqT[:, pr]).astype(bf),
            "wkt": np.ascontiguousarray(wkT[:, pr]).astype(bf),
            "wvt": np.ascontiguousarray(wvT[:, pr]).astype(bf),
            "wos": np.ascontiguousarray(woT[pr, :]).astype(bf),
            "bqs": np.ascontiguousarray(bq[pr].reshape(128, 1)),
            "bks": np.ascontiguousarray(bk[pr].reshape(128, 1)),
            "bvb": np.ascontiguousarray(
                np.broadcast_to(bv[pr][None, :], (128, 128))
            ),
            "bob": np.ascontiguousarray(
                np.broadcast_to(bo[None, :] * 0.25, (128, D))
            ),
        }
        in_maps.append(m)
    return in_maps


def assemble(results):
    out = np.empty((B, S, D), dtype=np.float32)
    for b in range(B):
        acc = results[b * PAIRS]["out"].astype(np.float32, copy=True)
        for hp in range(1, PAIRS):
            acc += results[b * PAIRS + hp]["out"]
        out[b] = acc
    return out


def kernel(x, Wq, bq, Wk, bk, Wv, bv, Wo, bo):
    in_maps = prepare_in_maps(x, Wq, bq, Wk, bk, Wv, bv, Wo, bo)
    nc = _get_program()
    res = run_bass_kernel_spmd(nc, in_maps, core_ids=list(range(NCORES)))
    return assemble(res.results)


# revision 24
# speedup vs baseline: 1.0520x; 1.0520x over previous
"""Multi-head attention (B=2, S=4096, D=512, H=8) on 8 TRN2 NeuronCores.

Sharding: (batch, head-pair) tensor parallel. Core i handles batch i//4
and heads 2*(i%4), 2*(i%4)+1. Each core computes Q/K/V projections only
for its two heads (1/4 of the projection work, no redundancy), full
S x S attention for those heads over all 4096 queries, and a PARTIAL
output projection out_partial = aot_pair^T @ Wo_pair + bo/4. The host
sums the 4 partials per batch (f32) -- no device collectives.

Per-core device pipeline (d-major transposed layout, bf16 matmuls):
  1. Transposing DMAs load x^T [d, t] in 4 segments of 1024 t.
  2. Q^T/K^T = Wpair x^T (f=512 matmuls, 128-row output = both heads);
     V natural [t, pair-dv] with a ones-column per head (V_aug).
  3. Per (q-tile 512, k-chunk 128): 2 row-packed score matmuls
     (c=64, heads at PE rows 0-63/64-127 run concurrently), one ACT
     exp [128,1024] psum->sbuf (scale=1/8), 2 attn@V matmuls
     lhsT=[V_h|1] [128,65] -> po [65,512]; row 64 accumulates the
     softmax denominator. scores/exp for k+1 are emitted before attn@V
     of k (software pipeline) so ACT -- the bottleneck engine -- is
     never starved.
  4. Normalize per q-tile: copy numerators into one partition-aligned
     [128, 512] sbuf tile and denominators to partition-0 rows (custom
     DVE ops require base partition 0), fast-approx reciprocal, two
     fp16 rank-1 broadcast matmuls into one psum bank, one aligned
     scalar_tensor_tensor multiply -> aot [128(2 heads' d), 512].
  5. Partial output projection: one c=128 matmul per 128-row t-chunk
     (both heads contracted at once), + bo/4, DMA out f32.

Steady state is ACT(exp)-bound: 256 instrs x ~1.0us ~= 260us; PE has
~80us of slack which absorbs projections and normalization.
"""

import numpy as np
import ml_dtypes

import concourse.bass as bass
import concourse.tile as tile
from concourse import bacc, mybir
from concourse.bass_utils import run_bass_kernel_spmd

F32 = mybir.dt.float32
F32R = mybir.dt.float32r
FP16 = mybir.dt.float16
BF16 = mybir.dt.bfloat16
MUL = mybir.AluOpType.mult

B, S, D, H = 2, 4096, 512, 8
HD = D // H  # 64
NCORES = 8
PAIRS = 4  # head-pairs; one per core (per batch)
IC = D // 128  # 4 contraction chunks over d_model
QT = 512  # q tile
NQT = S // QT  # 8
KCH = S // 128  # 32 k chunks
SEG = 1024  # t-columns per transposed DMA segment
NSEG = S // SEG  # 4


def _build_program():
    nc = bacc.Bacc(
        "TRN2",
        target_bir_lowering=False,
        debug=False,
        enable_asserts=False,
        num_devices=NCORES,
    )
    xt = nc.dram_tensor("xt", [D, S], BF16, kind="ExternalInput").ap()
    wqt = nc.dram_tensor("wqt", [D, 128], BF16, kind="ExternalInput").ap()
    wkt = nc.dram_tensor("wkt", [D, 128], BF16, kind="ExternalInput").ap()
    wvt = nc.dram_tensor("wvt", [D, 128], BF16, kind="ExternalInput").ap()
    wos = nc.dram_tensor("wos", [128, D], BF16, kind="ExternalInput").ap()
    bqs = nc.dram_tensor("bqs", [128, 1], F32, kind="ExternalInput").ap()
    bks = nc.dram_tensor("bks", [128, 1], F32, kind="ExternalInput").ap()
    bvb = nc.dram_tensor("bvb", [128, 128], F32, kind="ExternalInput").ap()
    bob = nc.dram_tensor("bob", [128, D], F32, kind="ExternalInput").ap()
    out = nc.dram_tensor("out", [S, D], F32, kind="ExternalOutput").ap()

    with tile.TileContext(nc) as tc:
        with (
            tc.tile_pool(name="consts", bufs=1) as consts,
            tc.tile_pool(name="persist", bufs=1) as persist,
            tc.tile_pool(name="pt", bufs=6) as pt_pool,
            tc.tile_pool(name="aot", bufs=2) as aot_pool,
            tc.tile_pool(name="osb", bufs=4) as osb_pool,
            tc.tile_pool(name="posb", bufs=4) as posb_pool,
            tc.tile_pool(name="small", bufs=4) as small_pool,
            # PSUM (8 banks): sc 2x2, po 2x1, acc 2x1
            tc.tile_pool(name="ps_sc", bufs=2, space="PSUM") as sc_pool,
            tc.tile_pool(name="ps_po", bufs=2, space="PSUM") as po_pool,
            tc.tile_pool(name="ps_acc", bufs=2, space="PSUM") as acc_pool,
        ):
            # ---- constants ----
            ones64f = consts.tile([1, HD], F32)
            nc.vector.memset(ones64f, 1.0)
            ones64 = consts.tile([1, HD], FP16)
            nc.vector.tensor_copy(ones64, ones64f)

            # ---- persistent activations ----
            xtks = [
                persist.tile([128, IC, SEG], BF16, name=f"xtk{s}")
                for s in range(NSEG)
            ]
            kt = persist.tile([128, S], BF16)  # K^T pair [dv, t]
            qt = persist.tile([128, S], BF16)  # Q^T pair
            # V_aug: [t-in-chunk, t-chunk, head-in-pair, 64 V cols + ones]
            v_sb = persist.tile([128, KCH, 2, HD + 1], BF16)
            nc.vector.memset(v_sb[:, :, :, HD : HD + 1], 1.0)

            # ---- DMAs: x^T is pre-transposed on the host, so these are
            # plain (non-XBAR) loads: seg 0, then weights, then segs 1-3.
            xtd = xt.rearrange("(c p) t -> p c t", p=128)
            nc.sync.dma_start(xtks[0][:, :, 0:512], xtd[:, :, 0:512])
            # everything the upfront projection units need (they only read
            # t 0..511) goes right after the first half-segment; the second
            # half and later segments follow
            wq_sb = consts.tile([128, IC, 128], BF16)
            nc.sync.dma_start(wq_sb, wqt.rearrange("(c p) o -> p c o", p=128))
            wk_sb = consts.tile([128, IC, 128], BF16)
            nc.sync.dma_start(wk_sb, wkt.rearrange("(c p) o -> p c o", p=128))
            bq_sb = consts.tile([128, 1], F32)
            nc.sync.dma_start(bq_sb, bqs)
            bk_sb = consts.tile([128, 1], F32)
            nc.sync.dma_start(bk_sb, bks)
            wv_sb = consts.tile([128, IC, 128], BF16)
            nc.sync.dma_start(wv_sb, wvt.rearrange("(c p) o -> p c o", p=128))
            bvb_sb = consts.tile([128, 128], F32)
            nc.sync.dma_start(bvb_sb, bvb)
            nc.sync.dma_start(xtks[0][:, :, 512:SEG], xtd[:, :, 512:SEG])
            bob_sb = consts.tile([128, D], F32)
            nc.sync.dma_start(bob_sb, bob)
            wo_sb = consts.tile([128, D], BF16)
            nc.sync.dma_start(wo_sb, wos)
            for s in range(1, NSEG):
                nc.sync.dma_start(xtks[s], xtd[:, :, s * SEG : (s + 1) * SEG])

            # ---- projection units ----
            def q_unit(tt):
                ps = acc_pool.tile([128, QT], F32, tag="acc", name=f"q{tt}")
                s, ss = divmod(tt, 2)
                for i in range(IC):
                    nc.tensor.matmul(
                        ps,
                        wq_sb[:, i, :],
                        xtks[s][:, i, ss * QT : (ss + 1) * QT],
                        start=(i == 0),
                        stop=(i == IC - 1),
                    )
                nc.vector.tensor_scalar_add(
                    qt[:, tt * QT : (tt + 1) * QT], ps, bq_sb[:, 0:1]
                )

            def k_unit(tt, lo=0, hi=QT):
                ps = acc_pool.tile(
                    [128, hi - lo], F32, tag="acc", name=f"k{tt}_{lo}"
                )
                s, ss = divmod(tt, 2)
                for i in range(IC):
                    nc.tensor.matmul(
                        ps,
                        wk_sb[:, i, :],
                        xtks[s][:, i, ss * QT + lo : ss * QT + hi],
                        start=(i == 0),
                        stop=(i == IC - 1),
                    )
                nc.vector.tensor_scalar_add(
                    kt[:, tt * QT + lo : tt * QT + hi], ps, bk_sb[:, 0:1]
                )

            def v_unit(j):
                # V rows for t-chunk j, both heads: [128 t, 128 dv] + bias
                ps = acc_pool.tile([128, 128], F32, tag="acc", name=f"v{j}")
                s, jj = divmod(j, 8)
                for i in range(IC):
                    nc.tensor.matmul(
                        ps,
                        xtks[s][:, i, jj * 128 : (jj + 1) * 128],
                        wv_sb[:, i, :],
                        start=(i == 0),
                        stop=(i == IC - 1),
                    )
                nc.vector.tensor_add(
                    v_sb[:, j, :, 0:HD],
                    ps.rearrange("p (h d) -> p h d", h=2),
                    bvb_sb.rearrange("p (h d) -> p h d", h=2),
                )

            # upfront: just enough for attention (qi=0) to start; K tile 0
            # is split so chunk 0's scores wait only on a 128-col mini-unit
            q_unit(0)
            k_unit(0, 0, 128)
            v_unit(0)
            k_unit(0, 128, QT)
            v_unit(1)
            k_unit(1)
            v_unit(2)
            v_unit(3)
            pending = []
            for u in (4, 5, 6, 7):
                pending.append(lambda j=u: v_unit(j))
            pending.append(lambda: q_unit(1))
            for tt in range(2, 8):  # k segs with their v chunks
                pending.append(lambda tt=tt: k_unit(tt))
                for j in range(4 * tt, 4 * tt + 4):
                    pending.append(lambda j=j: v_unit(j))
            for tt in range(2, 8):
                pending.append(lambda tt=tt: q_unit(tt))

            pending_slow = []

            aots = {}

            def norm_unit(qi, posbN, recs):
                pb2 = acc_pool.tile([128, QT], F32, tag="acc", name=f"pb{qi}")
                nc.tensor.matmul(
                    pb2[0:HD, :], ones64, recs[0], start=True, stop=True
                )
                nc.tensor.matmul(
                    pb2[HD:128, :], ones64, recs[1], start=True, stop=True
                )
                nc.vector.scalar_tensor_tensor(
                    aots[qi], pb2, 1.0, posbN, op0=MUL, op1=MUL
                )

            def fin_unit(qi, t4):
                ps = acc_pool.tile([128, D], F32, tag="acc", name=f"f{qi}_{t4}")
                nc.tensor.matmul(
                    ps,
                    aots[qi][:, t4 * 128 : (t4 + 1) * 128],
                    wo_sb,
                    start=True,
                    stop=True,
                )
                osb = osb_pool.tile([128, D], F32, tag="osb")
                nc.vector.tensor_add(osb, ps, bob_sb)
                t0 = qi * QT + t4 * 128
                nc.sync.dma_start(out[t0 : t0 + 128, :], osb)

            # ---- attention ----
            for qi in range(NQT):
                qs = qi * QT
                aots[qi] = aot_pool.tile(
                    [128, QT], BF16, tag="aot", name=f"aot{qi}"
                )
                po = [
                    po_pool.tile([HD + 1, QT], F32, tag="po", name=f"po{qi}_{hh}")
                    for hh in range(2)
                ]

                def scores_exp(k, qs=qs):
                    pss = sc_pool.tile([128, 2, QT], F32, tag="sc")
                    for hh in range(2):
                        off = hh * HD
                        nc.tensor.matmul(
                            pss[:, hh, :],
                            kt[off : off + HD, k * 128 : (k + 1) * 128],
                            qt[off : off + HD, qs : qs + QT],
                            start=True,
                            stop=True,
                        )
                    ptile = pt_pool.tile([128, 2, QT], BF16, tag="pt")
                    nc.scalar.activation(
                        ptile, pss, mybir.ActivationFunctionType.Exp,
                        scale=1.0 / np.sqrt(HD),
                    )
                    return ptile

                # software pipeline: scores/exp for k+1 before attn@V of k
                ptile = scores_exp(0)
                for k in range(KCH):
                    it = qi * KCH + k
                    nxt = scores_exp(k + 1) if k + 1 < KCH else None
                    for hh in range(2):
                        nc.tensor.matmul(
                            po[hh],
                            v_sb[:, k, hh, :],
                            ptile[:, hh, :],
                            start=(k == 0),
                            stop=(k == KCH - 1),
                        )
                    ptile = nxt
                    # drain deferred work into PE's slack (~1.5 units/iter
                    # keeps proj ahead of its deadlines without bursts that
                    # starve ACT)
                    if it >= 1 and pending and (
                        len(pending) > 6 or it % 4 == 2
                    ):
                        pending.pop(0)()
                        if it % 2 == 0 and pending and len(pending) > 6:
                            pending.pop(0)()
                    elif it % 5 == 0 and pending_slow:
                        pending_slow.pop(0)()

                # free po banks fast; defer the slow normalize + fin chain.
                # For the last q-tile, the numerator copies run on ACT (idle
                # after the final exp) in parallel with the DVE reciprocal
                # chain to shorten the tail.
                posbN = posb_pool.tile([128, QT], F32, tag="posb", name=f"posb{qi}")
                last = qi == NQT - 1
                if not last:
                    nc.vector.tensor_copy(posbN[0:HD, :], po[0][0:HD, :])
                    nc.vector.tensor_copy(posbN[HD : 2 * HD, :], po[1][0:HD, :])
                recs = []
                for hh in range(2):
                    db = small_pool.tile([1, QT], F32, tag="db")
                    if last and hh == 1:
                        # tail: ACT takes one denominator copy so the two
                        # reciprocal chains overlap across engines
                        nc.scalar.copy(db, po[hh][HD : HD + 1, :])
                    else:
                        nc.vector.tensor_copy(db, po[hh][HD : HD + 1, :])
                    recf = small_pool.tile([1, QT], F32, tag="recf")
                    nc.vector.reciprocal_approx_fast(recf, db)
                    rec = small_pool.tile([1, QT], FP16, tag="rec")
                    if last:
                        nc.scalar.copy(rec, recf)
                    else:
                        nc.vector.tensor_copy(rec, recf)
                    recs.append(rec)
                if last:
                    nc.scalar.copy(posbN[0:HD, :], po[0][0:HD, :])
                    nc.scalar.copy(posbN[HD : 2 * HD, :], po[1][0:HD, :])
                pending_slow.append(
                    lambda qi=qi, posbN=posbN, recs=recs: norm_unit(qi, posbN, recs)
                )
                pending_slow.extend(
                    lambda qi=qi, t4=t4: fin_unit(qi, t4) for t4 in range(4)
                )

            for u in pending + pending_slow:
                u()

    nc.compile()
    return nc


_NC_CACHE = None


def _get_program():
    global _NC_CACHE
    if _NC_CACHE is None:
        _NC_CACHE = _build_program()
    return _NC_CACHE


def prepare_in_maps(x, Wq, bq, Wk, bk, Wv, bv, Wo, bo):
    bf = ml_dtypes.bfloat16
    x = np.ascontiguousarray(np.asarray(x, dtype=np.float32)).astype(bf)
    wqT = np.asarray(Wq, np.float32).T  # [D in, D out-rows]
    wkT = np.asarray(Wk, np.float32).T
    wvT = np.asarray(Wv, np.float32).T
    woT = np.asarray(Wo, np.float32).T  # [D dv, D out]
    bq = np.asarray(bq, np.float32)
    bk = np.asarray(bk, np.float32)
    bv = np.asarray(bv, np.float32)
    bo = np.asarray(bo, np.float32)
    in_maps = []
    for core in range(NCORES):
        b = core // PAIRS
        hp = core % PAIRS
        pr = slice(hp * 128, (hp + 1) * 128)
        m = {
            "xt": np.ascontiguousarray(x[b].T),
            "wqt": np.ascontiguousarray(wqT[:, pr]).astype(bf),
            "wkt": np.ascontiguousarray(wkT[:, pr]).astype(bf),
            "wvt": np.ascontiguousarray(wvT[:, pr]).astype(bf),
            "wos": np.ascontiguousarray(woT[pr, :]).astype(bf),
            "bqs": np.ascontiguousarray(bq[pr].reshape(128, 1)),
            "bks": np.ascontiguousarray(bk[pr].reshape(128, 1)),
            "bvb": np.ascontiguousarray(
                np.broadcast_to(bv[pr][None, :], (128, 128))
            ),
            "bob": np.ascontiguousarray(
                np.broadcast_to(bo[None, :] * 0.25, (128, D))
            ),
        }
        in_maps.append(m)
    return in_maps


def assemble(results):
    out = np.empty((B, S, D), dtype=np.float32)
    for b in range(B):
        acc = results[b * PAIRS]["out"].astype(np.float32, copy=True)
        for hp in range(1, PAIRS):
            acc += results[b * PAIRS + hp]["out"]
        out[b] = acc
    return out


def kernel(x, Wq, bq, Wk, bk, Wv, bv, Wo, bo):
    in_maps = prepare_in_maps(x, Wq, bq, Wk, bk, Wv, bv, Wo, bo)
    nc = _get_program()
    res = run_bass_kernel_spmd(nc, in_maps, core_ids=list(range(NCORES)))
    return assemble(res.results)
